# revision 5
# baseline (speedup 1.0000x reference)
"""CrossNet (DCN) forward on 8 Trainium2 NeuronCores.

Reference computation (L=6 cross layers):
    x0 = inputs                                  # [B, D]
    x_{i+1} = x0 * (x_i @ w_i) + b_i + x_i       # w_i: [D,1], b_i: [D]

Algebraic factorization: by induction every layer state has the form
    x_i = x0 * c_i + beta_i
with a per-row scalar c_i ([B]) and a row-constant vector beta_i ([D]):
    beta_{i+1} = beta_i + b_i                    (data independent)
    c_{i+1} = c_i * (1 + u_i) + v_i,   u_i = x0 @ w_i,  v_i = beta_i @ w_i
    out = x0 * c_L + beta_L

So the whole network is one [B,D]@[D,6] matvec batch (u), a tiny per-row
recurrence, and one final scale-add over [B,D] — HBM traffic is read x0 +
write out, the memory-bound optimum.

Device mapping (per core, 2048 rows, tiles of 128 rows, groups of 2 tiles):

* one VectorE InstStreamTranspose (32x32 blocks, SBUF->SBUF) per
  quarter-tile: xb[(a,i), t, 32C+j] = x0[32a+j, 32C+i] in fp32, then one
  ScalarE ACTIVATE per quarter casts both tiles to fp16 (contiguous
  writes only — strided fp16 writes fault the HW). Single fp16 matmul
  precision (~1e-3 on the output) is far inside the 2e-2 gate, so no
  hi/lo split: that saves two full elementwise passes over x and half
  the matmul columns vs the split scheme.
* TensorE computes u with one fp16 matmul per packed column pair:
     lhsT[(a,i), (e,a',l)] = (a==a') * Wh[32(2*C2+e)+i, l]   ([128,48])
     rhs  = fp16 xb pair-slice, dims (j64, t)                 (N=128)
  accumulated into PSUM u_ps[(e,a,l), (j64,t)] over all C2. Slots where
  the rhs j64-half doesn't match the weight parity e hold garbage that
  the extraction never reads.
* u_ps halves are PE-transposed to [(2j+t), (e,a,l)]; VectorE sums the
  valid slices, runs the c recurrence, and a 0/1 selector matmul + 4
  partition-aligned diagonal copies deliver c as a [128,1] per-partition
  scalar per tile; ScalarE/VectorE apply out = x0 * c in place; DMA
  stores.

Sharding: data parallel over the batch dim (spec hint), params replicated.
"""

import numpy as np

B, D, L = 16384, 4096, 6
N_CORES = 8
B_SHARD = B // N_CORES   # 2048
P = 128                  # SBUF partitions
N_TILES = B_SHARD // P   # 16 row-tiles per core
TPG = 2                  # tiles per group
N_GRP = N_TILES // TPG   # 8 groups
N_C2 = D // 64           # 64 packed column pairs
QUARTER = D // 4         # stream-transpose granularity (1024)
C2PQ = N_C2 // 4         # packed pairs per quarter (16)
ML = 4 * L               # 24 = (a, l) columns per e slot
MW = 2 * ML              # 48 = (e, a, l) stationary columns

_prog_cache = {}


def _build_program(use_v: bool, use_b: bool):
    """Build the SPMD bass program for one core's shard."""
    from contextlib import ExitStack

    import concourse.bass as bass
    import concourse.tile as tile
    from concourse import bacc, mybir

    f32 = mybir.dt.float32
    f16 = mybir.dt.float16
    nc = bacc.Bacc("TRN2", target_bir_lowering=False, debug=False)

    x = nc.dram_tensor("x", [B_SHARD, D], f32, kind="ExternalInput").ap()
    # wb[(a,i), C2, (e,a',l)] = (a==a') * Wh[32*(2*C2+e) + i, l]
    wb = nc.dram_tensor("wb", [P, N_C2, MW], f16, kind="ExternalInput").ap()
    # sel[(j',t'), t, (a',j)] = (t'==t)*(j==j')  (only first 64 partitions used)
    sel = nc.dram_tensor("sel", [P, TPG, P], f32, kind="ExternalInput").ap()
    ident = nc.dram_tensor("ident", [P, P], f32, kind="ExternalInput").ap()
    if use_v:
        vb = nc.dram_tensor("vb", [P, ML], f32, kind="ExternalInput").ap()
    if use_b:
        bb = nc.dram_tensor("bb", [P, D], f32, kind="ExternalInput").ap()
    out = nc.dram_tensor("out", [B_SHARD, D], f32, kind="ExternalOutput").ap()

    with tile.TileContext(nc) as tc, ExitStack() as ctx:
        # Params load on the ScalarE HWDGE queue so tile loads (sync
        # queue) start at t=0; ScalarE's first compute comes much later.
        singles = ctx.enter_context(tc.tile_pool(name="singles", bufs=1))
        wb_sb = singles.tile([P, N_C2, MW], f16)
        nc.scalar.dma_start(wb_sb[:], wb[:])
        sel_sb = singles.tile([P, TPG, P], f32)
        nc.scalar.dma_start(sel_sb[:], sel[:])
        id_sb = singles.tile([P, P], f32)
        nc.scalar.dma_start(id_sb[:], ident[:])
        if use_v:
            vb_sb = singles.tile([P, ML], f32)
            nc.scalar.dma_start(vb_sb[:], vb[:])
        if use_b:
            bb_sb = singles.tile([P, D], f32)
            nc.scalar.dma_start(bb_sb[:], bb[:])

        xpool = ctx.enter_context(tc.tile_pool(name="xtile", bufs=3 * TPG))
        holdp = ctx.enter_context(tc.tile_pool(name="hold", bufs=1))
        xbq = ctx.enter_context(tc.tile_pool(name="xbq", bufs=3))
        packp = ctx.enter_context(tc.tile_pool(name="xpack", bufs=3))
        upsum = ctx.enter_context(
            tc.tile_pool(name="upsum", bufs=2, space=bass.MemorySpace.PSUM)
        )
        utps = ctx.enter_context(
            tc.tile_pool(name="utps", bufs=4, space=bass.MemorySpace.PSUM)
        )
        cps = ctx.enter_context(
            tc.tile_pool(name="cps", bufs=2, space=bass.MemorySpace.PSUM)
        )
        small = ctx.enter_context(tc.tile_pool(name="small", bufs=4))

        def compute_c(g, xts, c_dst):
            """u matvec + recurrence for one group; writes per-tile scale
            factors into c_dst[t] ([P,1] APs). Final scale NOT applied."""
            # u_ps[(e,a,l), (j64, t)] — see module docstring.
            u_ps = upsum.tile([P, 64, TPG], f32)
            for q in range(4):
                # Stream-transpose quarter q of each tile (fp32), then one
                # ScalarE cast to fp16 over both tiles (contiguous writes).
                xb = xbq.tile([P, TPG, QUARTER], f32, tag="xb32")
                for t in range(TPG):
                    nc.vector.transpose(
                        xb[:, t, :], xts[t][:, q * QUARTER : (q + 1) * QUARTER]
                    )
                xh = packp.tile([P, TPG, QUARTER], f16, tag="xh")
                nc.scalar.copy(xh[:], xb[:])
                for cl in range(C2PQ):
                    c2 = q * C2PQ + cl
                    rhs = xh[:, :, 64 * cl : 64 * (cl + 1)].transpose([0, 2, 1])
                    nc.tensor.matmul(
                        u_ps[0:MW, :, :],
                        lhsT=wb_sb[:, c2, :],
                        rhs=rhs,
                        start=(c2 == 0),
                        stop=(c2 == N_C2 - 1),
                        skip_group_check=True,
                    )

            # Transpose each 64-wide half of u_ps to [(2j+t), (e,a,l)].
            u_sb = small.tile([P, 64 * TPG], f32, tag="u_sb")
            nc.vector.tensor_copy(u_sb[0:MW, :], u_ps[0:MW, :, :])
            uts = []
            for q2 in range(2):
                utq = utps.tile([P, MW], f32, tag="utq")
                nc.tensor.transpose(
                    utq[0:64, :], u_sb[0:MW, 64 * q2 : 64 * (q2 + 1)], id_sb[0:MW, 0:MW]
                )
                uts.append(utq)

            # Valid slices: q2=0 (j64<32 -> e=0): cols [0:24]
            #               q2=1 (j64>=32 -> e=1): cols [24:48]
            # u1 = 1 + u, in [(2j+t), (a,l)].
            u1 = small.tile([P, ML], f32, tag="u1")
            nc.vector.tensor_scalar_add(u1[0:64, :], uts[0][0:64, 0:ML], 1.0)
            nc.vector.tensor_add(u1[0:64, :], u1[0:64, :], uts[1][0:64, ML : 2 * ML])

            # c = prod_l u1_l (+ v terms), in [(2j+t), (a,l)].
            u1v = u1[:].rearrange("p (a l) -> p a l", a=4, l=L)
            ctr = small.tile([P, 4], f32, tag="ctr")
            if use_v:
                vbv = vb_sb[:].rearrange("p (a l) -> p a l", a=4, l=L)
                tmp = small.tile([P, 4], f32, tag="ctmp")
                nc.vector.tensor_add(ctr[0:64, :], u1v[0:64, :, 0], vbv[0:64, :, 0])
                for i in range(1, L):
                    nc.vector.tensor_mul(tmp[0:64, :], ctr[0:64, :], u1v[0:64, :, i])
                    nc.vector.tensor_add(ctr[0:64, :], tmp[0:64, :], vbv[0:64, :, i])
            else:
                m3 = small.tile([P, 4, 3], f32, tag="m3")
                nc.vector.tensor_mul(m3[0:64, :, 0], u1v[0:64, :, 0], u1v[0:64, :, 1])
                nc.vector.tensor_mul(m3[0:64, :, 1], u1v[0:64, :, 2], u1v[0:64, :, 3])
                nc.vector.tensor_mul(m3[0:64, :, 2], u1v[0:64, :, 4], u1v[0:64, :, 5])
                nc.vector.tensor_mul(ctr[0:64, :], m3[0:64, :, 0], m3[0:64, :, 1])
                nc.vector.tensor_mul(ctr[0:64, :], ctr[0:64, :], m3[0:64, :, 2])

            for t in range(TPG):
                # j-broadcast: jb[(a',j), a] = ctr[2j+t, a] for all a'.
                jb_ps = cps.tile([P, 4], f32)
                nc.tensor.matmul(
                    jb_ps[:],
                    lhsT=sel_sb[0:64, t, :],
                    rhs=ctr[0:64, :],
                    start=True,
                    stop=True,
                )
                # Diagonal pick: c_col[32a+j] = jb[(a,j), a] — four
                # partition-aligned copies (no cross-partition movement).
                for a in range(4):
                    nc.vector.tensor_copy(
                        c_dst[t][32 * a : 32 * (a + 1), :],
                        jb_ps[32 * a : 32 * (a + 1), a : a + 1],
                    )

        def finalize(g, t, xt, c_col, engine_vec, halves=1):
            """out = x0 * c (+ beta) in place, then store via GpSimd DMA
            (separate queue: a store waiting on compute must not block
            tile loads on the sync queue)."""
            row0 = (g * TPG + t) * P
            H = D // halves
            for h in range(halves):
                cols = slice(h * H, (h + 1) * H)
                if engine_vec:
                    nc.vector.tensor_scalar_mul(xt[:, cols], xt[:, cols], c_col[:, 0:1])
                else:
                    nc.scalar.mul(xt[:, cols], xt[:, cols], c_col[:, 0:1])
                if use_b:
                    nc.vector.tensor_add(xt[:, cols], xt[:, cols], bb_sb[:, cols])
                nc.gpsimd.dma_start(out[row0 : row0 + P, cols], xt[:, cols])

        # Group 0: tiles load first and persist in a dedicated pool; its
        # scale factors are computed early but the final multiply + store
        # run at the very END — so the tail after the last tile load is
        # just one multiply+store instead of a full group pipeline.
        g0_tiles = []
        for t in range(TPG):
            xt = holdp.tile([P, D], f32, tag=f"g0x{t}")
            nc.sync.dma_start(xt[:], x[t * P : (t + 1) * P, :])
            g0_tiles.append(xt)
        c_hold = holdp.tile([P, TPG], f32)
        compute_c(0, g0_tiles, [c_hold[:, t : t + 1] for t in range(TPG)])

        for g in range(1, N_GRP):
            xts = []
            for t in range(TPG):
                xt = xpool.tile([P, D], f32, tag="xtile")
                nc.sync.dma_start(xt[:], x[(g * TPG + t) * P : (g * TPG + t + 1) * P, :])
                xts.append(xt)
            c_cols = []
            for t in range(TPG):
                c_col = small.tile([P, 1], f32, tag=f"c_col{t}")
                c_cols.append(c_col)
            compute_c(g, xts, [c[:, 0:1] for c in c_cols])
            for t in range(TPG):
                finalize(g, t, xts[t], c_cols[t], engine_vec=(t % 2 == 1))

        # Deferred group-0 finalize: halves pipelined to shrink the tail.
        for t in range(TPG):
            finalize(0, t, g0_tiles[t], c_hold[:, t : t + 1], engine_vec=(t % 2 == 1),
                     halves=2)

    nc.compile()
    return nc


def _get_program(use_v: bool, use_b: bool):
    key = (use_v, use_b)
    if key not in _prog_cache:
        _prog_cache[key] = _build_program(use_v, use_b)
    return _prog_cache[key]


# test.py reads this after a traced run to get exec_time_ns etc.
_last_results = None


def _host_prep(w_np: np.ndarray, b_np: np.ndarray):
    """Derive the device-side parameter tensors."""
    W = w_np[:, :, 0].T.astype(np.float32)  # [D, L]
    Wh = W.astype(np.float16)

    # wb[(a,i), C2, (e,a',l)] = (a==a') * Wh[32*(2*C2+e)+i, l]
    wb = np.zeros((P, N_C2, MW), dtype=np.float16)
    Wc = Wh.reshape(N_C2, 2, 32, L)  # [C2, e, i, l]
    for e in range(2):
        for a in range(4):
            wb[32 * a : 32 * (a + 1), :, e * ML + a * L : e * ML + (a + 1) * L] = (
                Wc[:, e].transpose(1, 0, 2)
            )

    # sel[(j',t'), t, (a',j)] = (t'==t) * (j==j'), partitions p = TPG*j' + t'
    p_idx = np.arange(P)
    jp, tp = p_idx // TPG, p_idx % TPG
    m_idx = np.arange(P)
    jm = m_idx % 32
    sel = np.zeros((P, TPG, P), dtype=np.float32)
    for t in range(TPG):
        sel[:, t, :] = ((tp[:, None] == t) & (jp[:, None] == jm[None, :])).astype(
            np.float32
        )

    ident = np.eye(P, dtype=np.float32)

    beta = np.zeros(D, dtype=np.float32)
    v = np.zeros(L, dtype=np.float32)
    for i in range(L):
        v[i] = float(beta @ W[:, i])
        beta = beta + b_np[i]
    return wb, sel, ident, v, beta


def kernel(inputs: np.ndarray, w: np.ndarray, b: np.ndarray) -> np.ndarray:
    import os

    from concourse.bass_utils import run_bass_kernel_spmd

    global _last_results

    x0 = np.ascontiguousarray(np.asarray(inputs, dtype=np.float32))
    w_np = np.asarray(w, dtype=np.float32)
    b_np = np.asarray(b, dtype=np.float32)
    assert x0.shape == (B, D) and w_np.shape == (L, D, 1) and b_np.shape == (L, D)

    wb, sel, ident, v, beta = _host_prep(w_np, b_np)

    use_v = bool(np.any(v != 0.0))
    use_b = bool(np.any(beta != 0.0))

    nc = _get_program(use_v, use_b)

    base = {"wb": wb, "sel": sel, "ident": ident}
    if use_v:
        # v broadcast to [(2j+t), (a,l)]: column (a,l) holds v[l].
        vbt = np.tile(v, 4)[None, :] * np.ones((P, 1), np.float32)
        base["vb"] = np.ascontiguousarray(vbt.astype(np.float32))
    if use_b:
        bb = np.broadcast_to(beta, (P, D)).astype(np.float32)
        base["bb"] = np.ascontiguousarray(bb)

    in_maps = [
        {**base, "x": x0[i * B_SHARD : (i + 1) * B_SHARD]} for i in range(N_CORES)
    ]

    trace = bool(int(os.environ.get("KERNEL_TRACE", "0")))
    res = run_bass_kernel_spmd(
        nc, in_maps, core_ids=list(range(N_CORES)), trace=trace
    )
    _last_results = res

    out = np.empty((B, D), dtype=np.float32)
    for i in range(N_CORES):
        out[i * B_SHARD : (i + 1) * B_SHARD] = res.results[i]["out"]
    return out


# revision 8
# speedup vs baseline: 1.1010x; 1.1010x over previous
"""CrossNet (DCN) forward on 8 Trainium2 NeuronCores.

Reference computation (L=6 cross layers):
    x0 = inputs                                  # [B, D]
    x_{i+1} = x0 * (x_i @ w_i) + b_i + x_i       # w_i: [D,1], b_i: [D]

Algebraic factorization: by induction every layer state has the form
    x_i = x0 * c_i + beta_i
with a per-row scalar c_i ([B]) and a row-constant vector beta_i ([D]):
    beta_{i+1} = beta_i + b_i                    (data independent)
    c_{i+1} = c_i * (1 + u_i) + v_i,   u_i = x0 @ w_i,  v_i = beta_i @ w_i
    out = x0 * c_L + beta_L

So the whole network is one [B,D]@[D,6] matvec batch (u), a tiny per-row
recurrence, and one final scale-add over [B,D] — HBM traffic is read x0 +
write out, the memory-bound optimum.

Device mapping (per core, 2048 rows, tiles of 128 rows, groups of 2 tiles):

* one VectorE InstStreamTranspose (32x32 blocks, SBUF->SBUF) per
  quarter-tile: xb[(a,i), t, 32C+j] = x0[32a+j, 32C+i] in fp32, then one
  ScalarE ACTIVATE per quarter casts both tiles to fp16 (contiguous
  writes only — strided fp16 writes fault the HW). Single fp16 matmul
  precision (~1e-3 on the output) is far inside the 2e-2 gate, so no
  hi/lo split: that saves two full elementwise passes over x and half
  the matmul columns vs the split scheme.
* TensorE computes u with one fp16 matmul per packed column pair:
     lhsT[(a,i), (e,a',l)] = (a==a') * Wh[32(2*C2+e)+i, l]   ([128,48])
     rhs  = fp16 xb pair-slice, dims (j64, t)                 (N=128)
  accumulated into PSUM u_ps[(e,a,l), (j64,t)] over all C2. Slots where
  the rhs j64-half doesn't match the weight parity e hold garbage that
  the extraction never reads.
* u_ps halves are PE-transposed to [(2j+t), (e,a,l)]; VectorE sums the
  valid slices, runs the c recurrence, and a 0/1 selector matmul + 4
  partition-aligned diagonal copies deliver c as a [128,1] per-partition
  scalar per tile; ScalarE/VectorE apply out = x0 * c in place; DMA
  stores.

Sharding: data parallel over the batch dim (spec hint), params replicated.
"""

import numpy as np

B, D, L = 16384, 4096, 6
N_CORES = 8
B_SHARD = B // N_CORES   # 2048
P = 128                  # SBUF partitions
N_TILES = B_SHARD // P   # 16 row-tiles per core
TPG = 2                  # tiles per group
N_GRP = N_TILES // TPG   # 8 groups
N_C2 = D // 64           # 64 packed column pairs
QUARTER = D // 4         # stream-transpose granularity (1024)
C2PQ = N_C2 // 4         # packed pairs per quarter (16)
ML = 4 * L               # 24 = (a, l) columns per e slot
MW = 2 * ML              # 48 = (e, a, l) stationary columns

_prog_cache = {}


def _build_program(use_v: bool, use_b: bool):
    """Build the SPMD bass program for one core's shard."""
    from contextlib import ExitStack

    import concourse.bass as bass
    import concourse.tile as tile
    from concourse import bacc, mybir

    f32 = mybir.dt.float32
    f16 = mybir.dt.float16
    nc = bacc.Bacc("TRN2", target_bir_lowering=False, debug=False)

    x = nc.dram_tensor("x", [B_SHARD, D], f32, kind="ExternalInput").ap()
    # wb[(a,i), C2, (e,a',l)] = (a==a') * Wh[32*(2*C2+e) + i, l]
    wb = nc.dram_tensor("wb", [P, N_C2, MW], f16, kind="ExternalInput").ap()
    # sel[(j',t'), t, (a',j)] = (t'==t)*(j==j')  (only first 64 partitions used)
    sel = nc.dram_tensor("sel", [P, TPG, P], f32, kind="ExternalInput").ap()
    ident = nc.dram_tensor("ident", [P, P], f32, kind="ExternalInput").ap()
    if use_v:
        vb = nc.dram_tensor("vb", [P, ML], f32, kind="ExternalInput").ap()
    if use_b:
        bb = nc.dram_tensor("bb", [P, D], f32, kind="ExternalInput").ap()
    out = nc.dram_tensor("out", [B_SHARD, D], f32, kind="ExternalOutput").ap()

    with tile.TileContext(nc) as tc, ExitStack() as ctx:
        # Params load on the ScalarE HWDGE queue so tile loads (sync
        # queue) start at t=0; ScalarE's first compute comes much later.
        singles = ctx.enter_context(tc.tile_pool(name="singles", bufs=1))
        wb_sb = singles.tile([P, N_C2, MW], f16)
        nc.scalar.dma_start(wb_sb[:], wb[:])
        sel_sb = singles.tile([P, TPG, P], f32)
        nc.scalar.dma_start(sel_sb[:], sel[:])
        id_sb = singles.tile([P, P], f32)
        nc.scalar.dma_start(id_sb[:], ident[:])
        if use_v:
            vb_sb = singles.tile([P, ML], f32)
            nc.scalar.dma_start(vb_sb[:], vb[:])
        if use_b:
            bb_sb = singles.tile([P, D], f32)
            nc.scalar.dma_start(bb_sb[:], bb[:])

        xpool = ctx.enter_context(tc.tile_pool(name="xtile", bufs=3 * TPG + 1))
        holdp = ctx.enter_context(tc.tile_pool(name="hold", bufs=1))
        xbq = ctx.enter_context(tc.tile_pool(name="xbq", bufs=2))
        packp = ctx.enter_context(tc.tile_pool(name="xpack", bufs=3))
        upsum = ctx.enter_context(
            tc.tile_pool(name="upsum", bufs=2, space=bass.MemorySpace.PSUM)
        )
        utps = ctx.enter_context(
            tc.tile_pool(name="utps", bufs=4, space=bass.MemorySpace.PSUM)
        )
        cps = ctx.enter_context(
            tc.tile_pool(name="cps", bufs=2, space=bass.MemorySpace.PSUM)
        )
        small = ctx.enter_context(tc.tile_pool(name="small", bufs=4))

        def compute_c(g, xts, c_dst):
            """u matvec + recurrence for one group; writes per-tile scale
            factors into c_dst[t] ([P,1] APs). Final scale NOT applied."""
            # u_ps[(e,a,l), (j64, t)] — see module docstring.
            u_ps = upsum.tile([P, 64, TPG], f32)
            for q in range(4):
                # Stream-transpose quarter q of each tile (fp32), then one
                # ScalarE cast to fp16 over both tiles (contiguous writes).
                xb = xbq.tile([P, TPG, QUARTER], f32, tag="xb32")
                for t in range(TPG):
                    nc.vector.transpose(
                        xb[:, t, :], xts[t][:, q * QUARTER : (q + 1) * QUARTER]
                    )
                xh = packp.tile([P, TPG, QUARTER], f16, tag="xh")
                nc.scalar.copy(xh[:], xb[:])
                for cl in range(C2PQ):
                    c2 = q * C2PQ + cl
                    rhs = xh[:, :, 64 * cl : 64 * (cl + 1)].transpose([0, 2, 1])
                    nc.tensor.matmul(
                        u_ps[0:MW, :, :],
                        lhsT=wb_sb[:, c2, :],
                        rhs=rhs,
                        start=(c2 == 0),
                        stop=(c2 == N_C2 - 1),
                        skip_group_check=True,
                    )

            # Transpose each 64-wide half of u_ps to [(2j+t), (e,a,l)].
            u_sb = small.tile([P, 64 * TPG], f32, tag="u_sb")
            nc.vector.tensor_copy(u_sb[0:MW, :], u_ps[0:MW, :, :])
            uts = []
            for q2 in range(2):
                utq = utps.tile([P, MW], f32, tag="utq")
                nc.tensor.transpose(
                    utq[0:64, :], u_sb[0:MW, 64 * q2 : 64 * (q2 + 1)], id_sb[0:MW, 0:MW]
                )
                uts.append(utq)

            # Valid slices: q2=0 (j64<32 -> e=0): cols [0:24]
            #               q2=1 (j64>=32 -> e=1): cols [24:48]
            # u1 = 1 + u, in [(2j+t), (a,l)].
            u1 = small.tile([P, ML], f32, tag="u1")
            nc.vector.tensor_scalar_add(u1[0:64, :], uts[0][0:64, 0:ML], 1.0)
            nc.vector.tensor_add(u1[0:64, :], u1[0:64, :], uts[1][0:64, ML : 2 * ML])

            # c = prod_l u1_l (+ v terms), in [(2j+t), (a,l)].
            u1v = u1[:].rearrange("p (a l) -> p a l", a=4, l=L)
            ctr = small.tile([P, 4], f32, tag="ctr")
            if use_v:
                vbv = vb_sb[:].rearrange("p (a l) -> p a l", a=4, l=L)
                tmp = small.tile([P, 4], f32, tag="ctmp")
                nc.vector.tensor_add(ctr[0:64, :], u1v[0:64, :, 0], vbv[0:64, :, 0])
                for i in range(1, L):
                    nc.vector.tensor_mul(tmp[0:64, :], ctr[0:64, :], u1v[0:64, :, i])
                    nc.vector.tensor_add(ctr[0:64, :], tmp[0:64, :], vbv[0:64, :, i])
            else:
                m3 = small.tile([P, 4, 3], f32, tag="m3")
                nc.vector.tensor_mul(m3[0:64, :, 0], u1v[0:64, :, 0], u1v[0:64, :, 1])
                nc.vector.tensor_mul(m3[0:64, :, 1], u1v[0:64, :, 2], u1v[0:64, :, 3])
                nc.vector.tensor_mul(m3[0:64, :, 2], u1v[0:64, :, 4], u1v[0:64, :, 5])
                nc.vector.tensor_mul(ctr[0:64, :], m3[0:64, :, 0], m3[0:64, :, 1])
                nc.vector.tensor_mul(ctr[0:64, :], ctr[0:64, :], m3[0:64, :, 2])

            for t in range(TPG):
                # j-broadcast: jb[(a',j), a] = ctr[2j+t, a] for all a'.
                jb_ps = cps.tile([P, 4], f32)
                nc.tensor.matmul(
                    jb_ps[:],
                    lhsT=sel_sb[0:64, t, :],
                    rhs=ctr[0:64, :],
                    start=True,
                    stop=True,
                )
                # Diagonal pick: c_col[32a+j] = jb[(a,j), a] — four
                # partition-aligned copies (no cross-partition movement).
                for a in range(4):
                    nc.vector.tensor_copy(
                        c_dst[t][32 * a : 32 * (a + 1), :],
                        jb_ps[32 * a : 32 * (a + 1), a : a + 1],
                    )

        def apply_scale(xt, c_col, engine_vec, cols):
            """xt[:, cols] *= c (+ beta), in place."""
            if engine_vec:
                nc.vector.tensor_scalar_mul(xt[:, cols], xt[:, cols], c_col[:, 0:1])
            else:
                nc.scalar.mul(xt[:, cols], xt[:, cols], c_col[:, 0:1])
            if use_b:
                nc.vector.tensor_add(xt[:, cols], xt[:, cols], bb_sb[:, cols])

        def finalize(g, t, xt, c_col, engine_vec, halves=1):
            """out = x0 * c (+ beta) in place, then store via GpSimd DMA
            (separate queue: a store waiting on compute must not block
            tile loads on the sync queue)."""
            row0 = (g * TPG + t) * P
            H = D // halves
            for h in range(halves):
                cols = slice(h * H, (h + 1) * H)
                apply_scale(xt, c_col, engine_vec, cols)
                nc.gpsimd.dma_start(out[row0 : row0 + P, cols], xt[:, cols])

        # Group 0: tiles load first and persist in a dedicated pool; its
        # scale factors are computed early but the final multiply + store
        # run at the very END — so the tail after the last tile load is
        # just one multiply+store instead of a full group pipeline.
        g0_tiles = []
        for t in range(TPG):
            xt = holdp.tile([P, D], f32, tag=f"g0x{t}")
            nc.sync.dma_start(xt[:], x[t * P : (t + 1) * P, :])
            g0_tiles.append(xt)
        c_hold = holdp.tile([P, TPG], f32)
        compute_c(0, g0_tiles, [c_hold[:, t : t + 1] for t in range(TPG)])

        for g in range(1, N_GRP):
            xts = []
            for t in range(TPG):
                xt = xpool.tile([P, D], f32, tag="xtile")
                nc.sync.dma_start(xt[:], x[(g * TPG + t) * P : (g * TPG + t + 1) * P, :])
                xts.append(xt)
            c_cols = []
            for t in range(TPG):
                c_col = small.tile([P, 1], f32, tag=f"c_col{t}")
                c_cols.append(c_col)
            compute_c(g, xts, [c[:, 0:1] for c in c_cols])
            for t in range(TPG):
                finalize(g, t, xts[t], c_cols[t], engine_vec=(t % 2 == 1))
            if g == N_GRP - 3:
                # Pre-scale group 0 in place while Vector/Scalar have
                # slack, so its end-of-program stores fire immediately.
                for t in range(TPG):
                    apply_scale(g0_tiles[t], c_hold[:, t : t + 1],
                                engine_vec=(t % 2 == 1), cols=slice(0, D))

        # Deferred group-0 stores: the tail after the last tile load is
        # just these DMAs (data already scaled), split for drain overlap.
        for t in range(TPG):
            row0 = t * P
            for h in range(2):
                cols = slice(h * (D // 2), (h + 1) * (D // 2))
                nc.gpsimd.dma_start(out[row0 : row0 + P, cols], g0_tiles[t][:, cols])

    nc.compile()
    return nc


def _get_program(use_v: bool, use_b: bool):
    key = (use_v, use_b)
    if key not in _prog_cache:
        _prog_cache[key] = _build_program(use_v, use_b)
    return _prog_cache[key]


# test.py reads this after a traced run to get exec_time_ns etc.
_last_results = None


def _host_prep(w_np: np.ndarray, b_np: np.ndarray):
    """Derive the device-side parameter tensors."""
    W = w_np[:, :, 0].T.astype(np.float32)  # [D, L]
    Wh = W.astype(np.float16)

    # wb[(a,i), C2, (e,a',l)] = (a==a') * Wh[32*(2*C2+e)+i, l]
    wb = np.zeros((P, N_C2, MW), dtype=np.float16)
    Wc = Wh.reshape(N_C2, 2, 32, L)  # [C2, e, i, l]
    for e in range(2):
        for a in range(4):
            wb[32 * a : 32 * (a + 1), :, e * ML + a * L : e * ML + (a + 1) * L] = (
                Wc[:, e].transpose(1, 0, 2)
            )

    # sel[(j',t'), t, (a',j)] = (t'==t) * (j==j'), partitions p = TPG*j' + t'
    p_idx = np.arange(P)
    jp, tp = p_idx // TPG, p_idx % TPG
    m_idx = np.arange(P)
    jm = m_idx % 32
    sel = np.zeros((P, TPG, P), dtype=np.float32)
    for t in range(TPG):
        sel[:, t, :] = ((tp[:, None] == t) & (jp[:, None] == jm[None, :])).astype(
            np.float32
        )

    ident = np.eye(P, dtype=np.float32)

    beta = np.zeros(D, dtype=np.float32)
    v = np.zeros(L, dtype=np.float32)
    for i in range(L):
        v[i] = float(beta @ W[:, i])
        beta = beta + b_np[i]
    return wb, sel, ident, v, beta


def kernel(inputs: np.ndarray, w: np.ndarray, b: np.ndarray) -> np.ndarray:
    import os

    from concourse.bass_utils import run_bass_kernel_spmd

    global _last_results

    x0 = np.ascontiguousarray(np.asarray(inputs, dtype=np.float32))
    w_np = np.asarray(w, dtype=np.float32)
    b_np = np.asarray(b, dtype=np.float32)
    assert x0.shape == (B, D) and w_np.shape == (L, D, 1) and b_np.shape == (L, D)

    wb, sel, ident, v, beta = _host_prep(w_np, b_np)

    use_v = bool(np.any(v != 0.0))
    use_b = bool(np.any(beta != 0.0))

    nc = _get_program(use_v, use_b)

    base = {"wb": wb, "sel": sel, "ident": ident}
    if use_v:
        # v broadcast to [(2j+t), (a,l)]: column (a,l) holds v[l].
        vbt = np.tile(v, 4)[None, :] * np.ones((P, 1), np.float32)
        base["vb"] = np.ascontiguousarray(vbt.astype(np.float32))
    if use_b:
        bb = np.broadcast_to(beta, (P, D)).astype(np.float32)
        base["bb"] = np.ascontiguousarray(bb)

    in_maps = [
        {**base, "x": x0[i * B_SHARD : (i + 1) * B_SHARD]} for i in range(N_CORES)
    ]

    trace = bool(int(os.environ.get("KERNEL_TRACE", "0")))
    res = run_bass_kernel_spmd(
        nc, in_maps, core_ids=list(range(N_CORES)), trace=trace
    )
    _last_results = res

    out = np.empty((B, D), dtype=np.float32)
    for i in range(N_CORES):
        out[i * B_SHARD : (i + 1) * B_SHARD] = res.results[i]["out"]
    return out
